# revision 40
# baseline (speedup 1.0000x reference)
"""3-layer MinGRU (LN -> x@W -> Heinsen scan -> residual) on 8 Trainium2 NeuronCores.

Sharding: data-parallel over batch B=8 (one sample per core); weights replicated.

Per-core layout: everything transposed — activations live as [D(partitions), S(free)]
so the recurrence h_t = (1-z_t)*h_{t-1} + z_t*g(h~_t) maps onto the DVE
tensor_tensor_scan instruction (scan along the free dim, one channel per lane).
The projection x_n @ W runs on the PE with W-tiles stationary and x_n^T moving,
in float32r (full-rate fp32 matmul mode, ~tf32 precision). LayerNorm stats
(mean / -var) are computed with 1/D-ones matmuls over the partition (D)
dimension and broadcast back through a DRAM-bounce DMA with a
partition-stride-0 access pattern; rstd = exp(-0.5*ln(var+eps)) keeps Ln/Exp
in one ACT table set. The minGRU activation uses the exact identity
g(x) = max(sigmoid(x), x+0.5), one DVE scalar_tensor_tensor op.
LN gamma/beta and the linear bias b are folded on the host into W' = diag(gamma)W
and b' = beta@W + b (exact here since gamma=1, beta=0, b=0).

Pipelining: the layer-0 stats prepass is emitted interleaved one chunk ahead of
layer 0's main chunks and shares its x tiles with them; the LN-apply stage
(x load, mean/rstd broadcast, center/scale) is hoisted one chunk ahead of the
compute loop so the in-order DVE stream produces xn(c+1) before chunk c's scan
tail; each next layer's rstd rows and weight loads are emitted before the
current layer's last chunk; the bias pack is stored pre-transposed so its DMA
is contiguous, and the layer-0 weight load is enqueued after the first x chunk.
Steady state runs at ~90-100% PE occupancy (cost-model timeline), ~844 us/core
predicted vs a 772 us PE-busy floor.
"""

import numpy as np

B, S, D, L = 8, 4096, 1024, 3
C = 512                 # s-chunk (moving-dim of each matmul)
CH = S // C             # chunks per layer
KT = D // 128           # contraction k-tiles
ET = (2 * D) // 128     # output e-tiles (16: 8 gate + 8 hidden)
LN_EPS = 1e-5

_cache = {}

# tuning knobs (read at _build time)
CFG = {
    "gh_bufs": 6, "st_bufs": 1, "x_bufs": 17, "xn_bufs": 13,
    "gout_bufs": 4, "h_bufs": 2, "out_bufs": 2, "mid_bufs": 2, "bc_bufs": 2,
    "residual_engine": "vector", "c1_engine": "vector",
    "z_engine": "scalar", "hidden_first": True, "skinny_bufs": 1, "carry_bufs": 2,
    "w_bufs": 9, "share_x": True, "pre_sq_engine": "scalar", "sq_engine": "scalar",
}


def _build():
    import concourse.bacc as bacc
    import concourse.mybir as mybir
    import concourse.tile as tile
    from contextlib import ExitStack

    F32 = mybir.dt.float32
    F32R = mybir.dt.float32r
    AF = mybir.ActivationFunctionType
    ALU = mybir.AluOpType
    import concourse.bass as bass

    nc = bacc.Bacc("TRN2", target_bir_lowering=False)

    xT = nc.dram_tensor("xT", (D, S), F32R, kind="ExternalInput")
    wts = [nc.dram_tensor(f"w{l}", (D, 2 * D), F32R, kind="ExternalInput")
           for l in range(L)]
    # rows = (l, variant, e8): variants 0=-bg, 1=+bg, 2=bh, 3=bh+0.5
    biases = nc.dram_tensor("biases", (128, L * 4 * KT + 2), F32R, kind="ExternalInput")
    yT = nc.dram_tensor("yT", (D, S), F32R, kind="ExternalOutput")
    hlast = nc.dram_tensor("hlast", (L, D), F32R, kind="ExternalOutput")

    with ExitStack() as ctx:
        tc = ctx.enter_context(tile.TileContext(nc))
        singles = ctx.enter_context(tc.tile_pool(name="singles", bufs=1))
        wpool = ctx.enter_context(tc.tile_pool(name="w", bufs=CFG["w_bufs"]))
        xin = ctx.enter_context(tc.tile_pool(name="xin", bufs=CFG["x_bufs"]))
        bc = ctx.enter_context(tc.tile_pool(name="bc", bufs=CFG["bc_bufs"]))
        mid = ctx.enter_context(tc.tile_pool(name="mid", bufs=CFG["mid_bufs"]))
        xnp = ctx.enter_context(tc.tile_pool(name="xn", bufs=CFG["xn_bufs"]))
        gout = ctx.enter_context(tc.tile_pool(name="gout", bufs=CFG["gout_bufs"]))
        hp = ctx.enter_context(tc.tile_pool(name="h", bufs=CFG["h_bufs"]))
        carryp = ctx.enter_context(tc.tile_pool(name="carry", bufs=CFG["carry_bufs"]))
        outp = ctx.enter_context(tc.tile_pool(name="out", bufs=CFG["out_bufs"]))
        skinny = ctx.enter_context(tc.tile_pool(name="skinny", bufs=CFG["skinny_bufs"]))
        rowp = ctx.enter_context(tc.tile_pool(name="rows", bufs=2))
        psum = ctx.enter_context(tc.tile_pool(name="psum", bufs=CFG["gh_bufs"], space="PSUM"))
        pstat = ctx.enter_context(tc.tile_pool(name="pstat", bufs=CFG["st_bufs"], space="PSUM"))
        dram = ctx.enter_context(tc.tile_pool(name="dram", bufs=1, space="DRAM"))

        bias_sb = singles.tile([128, L * 4 * KT + 2], F32R)
        nc.sync.dma_start(bias_sb, biases[:, :])
        inv_d = bias_sb[:, L * 4 * KT:L * 4 * KT + 1]
        eps_t = bias_sb[:, L * 4 * KT + 1:L * 4 * KT + 2]

        def bias_ap(l, v, j):
            col = (l * 4 + v) * KT + j
            return bias_sb[:, col:col + 1]

        scr = [dram.tile([D, S], F32R, tag=f"scr{i}", name=f"scr{i}") for i in range(2)]
        rows = dram.tile([3 * L, S], F32R, tag="rows")   # (l: mean, rstd, negvar)

        def bcast_ap(row, c0):
            src = rows[row:row + 1, c0:c0 + C]
            return bass.AP(tensor=src.tensor, offset=src.offset,
                           ap=[[0, 128], *src.ap[1:]])

        def stats_step(st, k, xt, xq):
            st0, st1 = st
            nc.tensor.matmul(st0[0:1, :], inv_d, xt,
                             start=(k == 0), stop=(k == KT - 1))
            nc.tensor.matmul(st1[0:1, :], inv_d, xq,
                             start=(k == 0), stop=(k == KT - 1))

        def stats_end(l, c, st, fused_rstd=False):
            """Finish mean / -var rows for (layer l, chunk c); write skinny to DRAM."""
            st0, st1 = st
            mrow = skinny.tile([1, C], F32R, tag="mrow")
            nc.vector.tensor_copy(mrow, st0[0:1, :])
            nc.sync.dma_start(rows[3 * l:3 * l + 1, c * C:(c + 1) * C], mrow)
            m2 = skinny.tile([1, C], F32R, tag="m2")
            nc.vector.tensor_tensor(out=m2, in0=mrow, in1=mrow, op=ALU.mult)
            negvar = skinny.tile([1, C], F32R, tag="negvar")
            nc.vector.tensor_tensor(out=negvar, in0=m2, in1=st1[0:1, :], op=ALU.subtract)
            if fused_rstd:
                nc.scalar.activation(negvar, negvar, AF.Ln, scale=-1.0, bias=eps_t[0:1, :])
                nc.scalar.activation(negvar, negvar, AF.Exp, scale=-0.5)
                nc.sync.dma_start(rows[3 * l + 1:3 * l + 2, c * C:(c + 1) * C], negvar)
            else:
                nc.sync.dma_start(rows[3 * l + 2:3 * l + 3, c * C:(c + 1) * C], negvar)

        def rstd_rows(l, lo, hi):
            """rstd = exp(-0.5*ln(var+eps)) over rows[negvar, lo:hi] -> rows[rstd, lo:hi]."""
            p = min(128, (hi - lo) // 4)
            nv = rowp.tile([p, (hi - lo) // p], F32R, tag="nv")
            src = rows[3 * l + 2:3 * l + 3, lo:hi].rearrange("one (p f) -> (one p) f", p=p)
            dst = rows[3 * l + 1:3 * l + 2, lo:hi].rearrange("one (p f) -> (one p) f", p=p)
            nc.sync.dma_start(nv, src)
            nc.scalar.activation(nv, nv, AF.Ln, scale=-1.0, bias=eps_t[:p, :])
            nc.scalar.activation(nv, nv, AF.Exp, scale=-0.5)
            nc.sync.dma_start(dst, nv)

        def load_weights(l):
            # per-k tiles; 1 spare pool slot lets the next layer's first
            # k-tile prefetch during this layer's tail
            w_sb = [wpool.tile([128, 2 * D], F32R, tag="wsb", name=f"wsb{l}_{k}")
                    for k in range(KT)]
            for k in range(KT):
                for q in range(4):
                    nc.sync.dma_start(
                        w_sb[k][:, q * 512:(q + 1) * 512],
                        wts[l][k * 128:(k + 1) * 128, q * 512:(q + 1) * 512])
            return w_sb

        def prepass_chunk(c):
            st = (pstat.tile([1, C], F32, tag="st0", name="st0"),
                  pstat.tile([1, C], F32, tag="st1", name="st1"))
            xts = []
            for j in range(KT):
                xt = xin.tile([128, C], F32R, tag="x")
                nc.sync.dma_start(xt, xT[j * 128:(j + 1) * 128, c * C:(c + 1) * C])
                xq = mid.tile([128, C], F32R, tag="oq")
                peng = CFG.get("pre_sq_engine", "vector")
                if peng == "scalar":
                    nc.scalar.activation(xq, xt, AF.Square)
                else:
                    getattr(nc, peng).tensor_tensor(out=xq, in0=xt, in1=xt, op=ALU.mult)
                stats_step(st, j, xt, xq)
                xts.append(xt)
            stats_end(0, c, st, fused_rstd=True)
            return xts

        def ln_apply(l, c, indram, xts_in=None):
            c0 = c * C
            if xts_in is not None:
                xts = xts_in
            else:
                xts = []
                for j in range(KT):
                    xt = xin.tile([128, C], F32R, tag="x")
                    nc.sync.dma_start(xt, indram[j * 128:(j + 1) * 128, c0:c0 + C])
                    xts.append(xt)
            mb = bc.tile([128, C], F32R, tag="mb")
            nc.sync.dma_start(mb, bcast_ap(3 * l, c0))
            rb = bc.tile([128, C], F32R, tag="rb")
            nc.sync.dma_start(rb, bcast_ap(3 * l + 1, c0))
            xns = []
            for j in range(KT):
                c1 = mid.tile([128, C], F32R, tag="c1")
                getattr(nc, CFG["c1_engine"]).tensor_tensor(out=c1, in0=xts[j], in1=mb, op=ALU.subtract)
                xn = xnp.tile([128, C], F32R, tag="xn")
                nc.vector.tensor_tensor(out=xn, in0=c1, in1=rb, op=ALU.mult)
                xns.append(xn)
            return xts, xns

        def main_chunk(l, c, indram, outdram, w_sb, carries, ln):
            if True:
                c0 = c * C
                xts, xns = ln

                st = (pstat.tile([1, C], F32, tag="st0", name="st0"),
                      pstat.tile([1, C], F32, tag="st1", name="st1")) if l + 1 < L else None
                for j in range(KT):
                    def gate_part(j=j):
                        pg = psum.tile([128, C], F32, tag="gh", name="pg")
                        for k in range(KT):
                            nc.tensor.matmul(pg, w_sb[k][:, j * 128:(j + 1) * 128],
                                             xns[k], start=(k == 0), stop=(k == KT - 1))
                        a = gout.tile([128, C], F32R, tag="a", name="a")
                        nc.scalar.activation(a, pg, AF.Sigmoid, scale=-1.0, bias=bias_ap(l, 0, j))
                        return pg, a
                    if not CFG["hidden_first"]:
                        pg, a = gate_part()
                    # hidden e-tile j
                    ph = psum.tile([128, C], F32, tag="gh")
                    for k in range(KT):
                        nc.tensor.matmul(ph, w_sb[k][:, D + j * 128:D + (j + 1) * 128],
                                         xns[k], start=(k == 0), stop=(k == KT - 1))
                    sh = gout.tile([128, C], F32R, tag="sh")
                    nc.scalar.activation(sh, ph, AF.Sigmoid, scale=1.0, bias=bias_ap(l, 2, j))
                    # g(x) = max(sigmoid(x), x + 0.5) exactly (curves cross at x=0)
                    g = gout.tile([128, C], F32R, tag="g")
                    nc.vector.scalar_tensor_tensor(g, ph, bias_ap(l, 3, j), sh,
                                                   op0=ALU.add, op1=ALU.max)
                    if CFG["hidden_first"]:
                        pg, a = gate_part()
                    z = gout.tile([128, C], F32R, tag="z")
                    if CFG["z_engine"] == "scalar":
                        nc.scalar.activation(z, pg, AF.Sigmoid, scale=1.0, bias=bias_ap(l, 1, j))
                    else:
                        getattr(nc, CFG["z_engine"]).tensor_scalar(
                            out=z, in0=a, scalar1=-1.0, scalar2=1.0,
                            op0=ALU.mult, op1=ALU.add)
                    bs = gout.tile([128, C], F32R, tag="bs")
                    nc.gpsimd.tensor_tensor(out=bs, in0=g, in1=z, op=ALU.mult)
                    h = hp.tile([128, C], F32R, tag="h")
                    init = 0.5 if c == 0 else carries[j][:, 0:1]
                    nc.vector.tensor_tensor_scan(h, a, bs, init,
                                                 op0=ALU.mult, op1=ALU.add)
                    cj = carryp.tile([128, 1], F32R, tag=f"cr{j}")
                    nc.vector.tensor_copy(cj, h[:, C - 1:C])
                    carries[j] = cj
                    ot = outp.tile([128, C], F32R, tag="o")
                    getattr(nc, CFG["residual_engine"]).tensor_tensor(out=ot, in0=h, in1=xts[j], op=ALU.add)
                    nc.sync.dma_start(outdram[j * 128:(j + 1) * 128, c0:c0 + C], ot)
                    if c == CH - 1:
                        nc.sync.dma_start(
                            hlast[l:l + 1, j * 128:(j + 1) * 128].rearrange("one p -> p one"),
                            cj[:, :])
                    if l + 1 < L:
                        xq = mid.tile([128, C], F32R, tag="oq")
                        eng = CFG.get("sq_engine", "scalar")
                        if eng == "scalar":
                            nc.scalar.activation(xq, ot, AF.Square)
                        else:
                            getattr(nc, eng).tensor_tensor(out=xq, in0=ot, in1=ot, op=ALU.mult)
                        stats_step(st, j, ot, xq)
                if l + 1 < L:
                    stats_end(l + 1, c, st)

        dram_io = [(xT, scr[0]), (scr[0], scr[1]), (scr[1], yT)]
        w_cur = None   # layer-0 weights loaded after the first x chunk is enqueued
        w_next = None
        pending = {}
        lns = {}
        for l in range(L):
            carries = [None] * KT
            for c in range(CH):
                if l == 0:
                    # prepass (layer-0 LN stats) runs one chunk ahead of the
                    # main pipeline; its x tiles are reused by the main chunk
                    if c == 0:
                        pending[0] = prepass_chunk(0)
                        w_cur = load_weights(0)
                        lns[0] = ln_apply(0, 0, dram_io[0][0], xts_in=pending.pop(0))
                    if c + 1 < CH:
                        pending[c + 1] = prepass_chunk(c + 1)
                        lns[c + 1] = ln_apply(0, c + 1, dram_io[0][0],
                                              xts_in=pending.pop(c + 1))
                else:
                    if c == 0:
                        lns[0] = ln_apply(l, 0, dram_io[l][0])
                    if c + 1 < CH:
                        lns[c + 1] = ln_apply(l, c + 1, dram_io[l][0])
                main_chunk(l, c, *dram_io[l], w_cur, carries, lns.pop(c))
                # emit the next layer's rstd + weight loads before this
                # layer's last chunk so the in-order ACT/DMA streams reach
                # them while the tail chunk is still computing
                if l + 1 < L and c == CH - 2:
                    rstd_rows(l + 1, 0, S - C)
                    w_next = load_weights(l + 1)
                if l + 1 < L and c == CH - 1:
                    rstd_rows(l + 1, S - C, S)
            w_cur = w_next


    nc.compile()
    return nc


def _get_nc():
    if "nc" not in _cache:
        _cache["nc"] = _build()
    return _cache["nc"]


def kernel(x, ln_gamma, ln_beta, W, b):
    from concourse.bass_utils import run_bass_kernel_spmd

    x = np.asarray(x, dtype=np.float32)
    ln_gamma = np.asarray(ln_gamma, dtype=np.float32)
    ln_beta = np.asarray(ln_beta, dtype=np.float32)
    W = np.asarray(W, dtype=np.float32)
    b = np.asarray(b, dtype=np.float32)

    # Fold LN affine + linear bias into W' and b' (host-side, exact for these fills)
    Wp = (W * ln_gamma[:, :, None]).astype(np.float32)           # (L, D, 2D)
    bp = (np.einsum("ld,lde->le", ln_beta.astype(np.float64),
                    W.astype(np.float64)) + b).astype(np.float32)  # (L, 2D)
    bg = bp[:, :D]     # gate bias
    bh = bp[:, D:]     # hidden bias

    # bias pack rows = (l, variant, e8) with variants (-bg, +bg, bh, bh+0.5)
    pack = np.zeros((L, 4, KT, 128), np.float32)
    for l in range(L):
        pack[l, 0] = (-bg[l]).reshape(KT, 128)
        pack[l, 1] = bg[l].reshape(KT, 128)
        pack[l, 2] = bh[l].reshape(KT, 128)
        pack[l, 3] = (bh[l] + 0.5).reshape(KT, 128)
    pack = pack.reshape(L * 4 * KT, 128)
    extra = np.stack([np.full(128, 1.0 / D, np.float32),
                      np.full(128, LN_EPS, np.float32)])
    pack = np.ascontiguousarray(np.concatenate([pack, extra], axis=0).T)  # (128, R)

    nc = _get_nc()
    in_maps = []
    for c in range(B):
        m = {"xT": np.ascontiguousarray(x[c].T), "biases": pack}
        for l in range(L):
            m[f"w{l}"] = Wp[l]
        in_maps.append(m)

    _cache["last_in_maps"] = in_maps
    res = run_bass_kernel_spmd(nc, in_maps, core_ids=list(range(B)),
                               **_cache.get("run_kwargs", {}))
    _cache["last_result"] = res

    out = np.stack([r["yT"].T for r in res.results])             # (B, S, D)
    hl = np.stack([r["hlast"] for r in res.results])             # (B, L, D)
    next_hidden = tuple(hl[:, l:l + 1, :] for l in range(L))     # L x (B, 1, D)
    return (out, *next_hidden)


if __name__ == "__main__":
    # quick self-check against a numpy reference (direct-space recurrence)
    rng = np.random.default_rng(0)
    x = rng.standard_normal((B, S, D)).astype(np.float32)
    W = (rng.standard_normal((L, D, 2 * D)) * 0.02).astype(np.float32)
    b = np.zeros((L, 2 * D), np.float32)
    gam = np.ones((L, D), np.float32)
    bet = np.zeros((L, D), np.float32)

    def ref_np(x, gam, bet, W, b):
        inp = x.astype(np.float64)
        nh = []
        for l in range(L):
            mu = inp.mean(-1, keepdims=True)
            var = inp.var(-1, keepdims=True)
            xn = (inp - mu) / np.sqrt(var + LN_EPS) * gam[l] + bet[l]
            gh = xn @ W[l].astype(np.float64) + b[l]
            gate, hid = gh[..., :D], gh[..., D:]
            zz = 1 / (1 + np.exp(-gate))
            gg = np.where(hid >= 0, hid + 0.5, 1 / (1 + np.exp(-hid)))
            h = np.full((B, D), 0.5)
            hs = np.zeros_like(gate)
            for t in range(S):
                h = (1 - zz[:, t]) * h + zz[:, t] * gg[:, t]
                hs[:, t] = h
            nh.append(hs[:, -1:].copy())
            inp = hs + inp
        return (inp, *nh)

    got = kernel(x, gam, bet, W, b)
    exp = ref_np(x, gam, bet, W, b)
    for i, (g, e) in enumerate(zip(got, exp)):
        scale = np.abs(e).max()
        err = np.abs(g - e)
        rel = err / (np.abs(e) + 1e-3 * scale)
        print(f"out[{i}]: shape={g.shape} absmax={err.max():.3e} scale={scale:.2f} "
              f"max_rel={rel.max():.3e} mean_rel={rel.mean():.3e}")
